# revision 15
# baseline (speedup 1.0000x reference)
"""3x3 valid conv (NCHW, stride 1) on 8 Trainium2 NeuronCores.

x: (16, 128, 64, 64) f32, weights: (256, 128, 3, 3) f32
-> out: (16, 256, 62, 62) f32

Data-parallel: 2 images per core, weights replicated. Per core the conv
is 9 shifted accumulated matmuls per output tile: contraction over
cin=128 (SBUF partitions), cout=256 split into two 128-partition PSUM
halves, free dim = 8 output rows x 62 cols = 496 (<= 512 fp32 PSUM bank).
Matmuls run in float32r (full PE rate, ~1e-4 relative error).

Input arrives as 10-row strips (one per 8-row output block) so the first
matmul starts ~2.6us after DMA begins; weights are laid out couth-major
and split into two DMAs for the same reason. Output stores go through
the scalar-engine HWDGE ring so they never queue behind input strips on
the sync ring. Taps iterate outer over quads of 4 row-blocks (4 PSUM
banks each, 8 banks double-buffered) so consecutive matmuls reuse the
same stationary weights where possible.
"""

import numpy as np

N_CORES = 8
IMGS_PER_CORE = 2
CIN = 128
COUT = 256
H = W = 64
OH = OW = 62
RPB = 8  # output rows per block

_NC_CACHE = []


def _build():
    import concourse.bacc as bacc
    import concourse.mybir as mybir
    import concourse.tile as tile

    f32r = mybir.dt.float32r
    f32 = mybir.dt.float32

    nc = bacc.Bacc("TRN2", target_bir_lowering=False, debug=False)
    x = nc.dram_tensor(
        "x", [IMGS_PER_CORE, CIN, H, W], f32r, kind="ExternalInput"
    ).ap()
    # w layout: [cin, (couth, tap, coutl)]; tap = 3*kh + kw
    w = nc.dram_tensor("w", [CIN, 2 * 9 * 128], f32r, kind="ExternalInput").ap()
    out = nc.dram_tensor(
        "out", [IMGS_PER_CORE, COUT, OH, OW], f32, kind="ExternalOutput"
    ).ap()

    with tile.TileContext(nc) as tc:
        with (
            tc.tile_pool(name="wp", bufs=1) as w_pool,
            tc.tile_pool(name="xs", bufs=1) as x_pool,
            tc.tile_pool(name="ost", bufs=6) as out_pool,
            tc.tile_pool(name="ps", bufs=8, space="PSUM") as ps_pool,
        ):
            # PE prewarm: dummy fp32 matmuls on memset scratch keep the PE
            # busy through the HAM window while input DMA streams in, so
            # real matmuls start at 2.4 GHz.
            scr = w_pool.tile([CIN, 128], mybir.dt.float32, tag="scr")
            nc.gpsimd.memset(scr[:], 0.0)
            wp = ps_pool.tile([128, RPB, OW], f32, name="wp", tag="pt")
            for _ in range(12):
                nc.tensor.matmul(
                    wp[:, :2, :], scr[:], scr[:, :124], start=True, stop=True
                )

            w_sb = w_pool.tile([CIN, 2 * 9 * 128], f32r, tag="w")
            strips = {}

            def load_strip(img, b, eng):
                r0 = RPB * b
                nrows = min(RPB + 2, H - r0)  # 10, last block 8
                st = x_pool.tile(
                    [CIN, RPB + 2, W], f32r, name=f"s{img}_{b}", tag=f"s{img}_{b}"
                )
                eng.dma_start(st[:, :nrows, :], x[img, :, r0 : r0 + nrows, :])
                strips[img, b] = st

            # weights on the sync ring; image-0 strips on the scalar ring so
            # both streams transfer in parallel (out-stores queue on scalar
            # only after ~18us, well past the strip prefetch)
            nc.sync.dma_start(w_sb[:, :128], w[:, :128])  # h0 tap0
            load_strip(0, 0, nc.scalar)
            nc.sync.dma_start(w_sb[:, 128:1152], w[:, 128:1152])  # h0 rest
            load_strip(0, 1, nc.scalar)
            nc.sync.dma_start(w_sb[:, 1152:], w[:, 1152:])  # h1
            for b in range(2, 8):
                load_strip(0, b, nc.scalar)
            for b in range(8):
                load_strip(1, b, nc.sync)

            first = True
            for img in range(IMGS_PER_CORE):
                for h in range(2):  # cout half
                    # First group runs block-by-block (needs only strip 0 to
                    # start); middle groups run taps outer over all 8 blocks
                    # (8 PSUM banks) so walrus ldw-opt dedupes weight loads;
                    # the final group goes block-by-block again so its
                    # copies/stores drain during compute instead of after
                    # the last matmul.
                    if first:
                        quads = [[0], [1], [2, 3], [4, 5, 6, 7]]
                        first = False
                    elif (img, h) == (IMGS_PER_CORE - 1, 1):
                        quads = [[0, 1, 2, 3], [4, 5], [6], [7]]
                    else:
                        quads = [[0, 1, 2, 3, 4, 5, 6, 7]]
                    for blocks in quads:
                        pts = {}
                        for t in range(9):
                            ki, kj = divmod(t, 3)
                            wsl = w_sb[:, h * 1152 + t * 128 : h * 1152 + t * 128 + 128]
                            for b in blocks:
                                R = min(RPB, OH - RPB * b)  # 8, last block 6
                                if t == 0:
                                    pts[b] = ps_pool.tile(
                                        [128, RPB, OW], f32, name="pt", tag="pt"
                                    )
                                nc.tensor.matmul(
                                    pts[b][:, :R, :],
                                    wsl,
                                    strips[img, b][:, ki : ki + R, kj : kj + OW],
                                    start=(t == 0),
                                    stop=(t == 8),
                                )
                        for b in blocks:
                            R = min(RPB, OH - RPB * b)
                            ot = out_pool.tile([128, RPB, OW], f32)
                            nc.vector.tensor_copy(ot[:, :R, :], pts[b][:, :R, :])
                            nc.scalar.dma_start(
                                out[img, h * 128 : h * 128 + 128, RPB * b : RPB * b + R, :],
                                ot[:, :R, :],
                            )
    nc.compile()
    return nc


def _get_nc():
    if not _NC_CACHE:
        _NC_CACHE.append(_build())
    return _NC_CACHE[0]


def _pack_weights(weights):
    # [cout, cin, kh, kw] -> [cin, couth, kh, kw, coutl] -> [cin, 2*9*128]
    wt = weights.reshape(2, 128, CIN, 3, 3).transpose(2, 0, 3, 4, 1)
    return np.ascontiguousarray(wt.reshape(CIN, 2 * 9 * 128))


def _ldw_opt_patch():
    """Enable walrus's LDWEIGHTS dedup pass (concourse pins it off) so
    back-to-back matmuls sharing a stationary operand skip the reload."""
    import contextlib

    from concourse import bass_utils as _bu

    @contextlib.contextmanager
    def _ctx():
        orig = _bu.run_command

        def patched(argv, **kw):
            argv = [
                "--enable-ldw-opt=true" if a == "--enable-ldw-opt=false" else a
                for a in argv
            ]
            return orig(argv, **kw)

        _bu.run_command = patched
        try:
            yield
        finally:
            _bu.run_command = orig

    return _ctx()


def kernel(x, weights):
    from concourse.bass_utils import run_bass_kernel_spmd

    x = np.ascontiguousarray(x, dtype=np.float32)
    weights = np.ascontiguousarray(weights, dtype=np.float32)
    w_l = _pack_weights(weights)

    nc = _get_nc()
    in_maps = [
        {"x": x[IMGS_PER_CORE * c : IMGS_PER_CORE * (c + 1)], "w": w_l}
        for c in range(N_CORES)
    ]
    with _ldw_opt_patch():
        res = run_bass_kernel_spmd(nc, in_maps, core_ids=list(range(N_CORES)))
    return np.concatenate([r["out"] for r in res.results], axis=0)
